# revision 5
# baseline (speedup 1.0000x reference)
"""Trainium2 Bass kernel for BlockChunkedActivityRoutedNet (v2: speculative L1).

Reference computation (B=4096, IN_F=4096, 8 chunks of 512, top-2 by mean|x|,
chunk-expert Linears 512->512, concat -> final Linear 1024->4096):

    xr = x.reshape(B, 8, 512)
    activities = mean(|xr|, axis=(0, 2))            # over the WHOLE batch
    i0, i1 = top2(activities)                        # descending
    h = concat(xr[:, i0] @ Wc[i0] + bc[i0], xr[:, i1] @ Wc[i1] + bc[i1])
    out = h @ W_final + b_final

Distribution: data-parallel over the batch across 8 NeuronCores (512 rows
each).  Per-chunk per-partition |x| partial sums are AllReduced as a [128, 8]
f32 tile; the partition reduce + top-2 run post-AR on every core identically.

v2 design (from the v1 trace: AR window = entry-barrier skew + ~25us
machinery; L2 at the GPIO-throttled PE rate ~262ns/MM = 67us; gather+L1
after the AR cost ~18us):
  - L1 runs SPECULATIVELY for ALL 8 chunks while the cores wait out the
    entry barrier + AllReduce (PE is idle there anyway).  hT[c][d] tiles for
    every chunk stay in SBUF; after routing, a 8-op DVE gated-sum per k-tile
    "selects" the two routed chunks (exact: gates are 1.0/0.0 in bf16).
  - x and W_chunks load through 4-row-packed DRAM views ([128, 2048] tiles,
    4KB packets) spread across sync+scalar HWDGE rings and the gpsimd SWDGE
    ring; x first (it gates the AR trigger), Wc behind it.
  - The AR carries [128, 8] per-partition sums so NO PE op sits in the
    trigger path (PE is busy with L1); partition-reduce happens post-AR.
  - W_final is host-repacked o-major ([o, p, kf, n]) so L2 streams output-
    column slabs: the o-outer loop consumes wfo[o] right as it arrives; no
    stall on the 8MB load.  Out rows DMA in half-rows after o=3 / o=7.
  - Post-AR warmup MMs (act_g-gated) re-warm the PE before L2.
"""

import numpy as np
import ml_dtypes

import concourse.bass as bass
import concourse.bacc as bacc
import concourse.mybir as mybir
from concourse.tile import TileContext
from concourse.bass_utils import run_bass_kernel_spmd
from concourse.masks import make_identity

dt = mybir.dt
P = 128

NUM_CHUNKS = 8
TOP_K = 2
IN_F = 4096
HID_F = 4096
OUT_F = 4096
B = 4096
CIN = IN_F // NUM_CHUNKS      # 512
COUT = HID_F // NUM_CHUNKS    # 512
N_CORES = 8
BS = B // N_CORES             # 512 rows per core

BT = BS // P                  # 4 batch tiles per core
DT_ = COUT // P               # 4 d-tiles per chunk
KF = TOP_K * DT_              # 8 k-tiles for the final matmul
OT = OUT_F // 512             # 8 output column tiles of 512
Q = 4                         # DRAM row packing for x / Wc views

_cache = {}


def _build():
    nc = bacc.Bacc(num_devices=N_CORES, name="chunk_routed_v2",
                   num_swdge_queues=4)

    xT = nc.dram_tensor("xT_shard", [IN_F, BS], dt.bfloat16,
                        kind="ExternalInput")
    Wc = nc.dram_tensor("W_chunks", [NUM_CHUNKS, CIN, COUT], dt.bfloat16,
                        kind="ExternalInput")
    bc_t = nc.dram_tensor("b_chunks", [NUM_CHUNKS, COUT], dt.float32,
                          kind="ExternalInput")
    Wfr = nc.dram_tensor("W_final_r", [KF * P, OUT_F], dt.bfloat16,
                         kind="ExternalInput")
    bf = nc.dram_tensor("b_final", [1, OUT_F], dt.float32, kind="ExternalInput")
    out = nc.dram_tensor("out_shard", [BS, OUT_F], dt.bfloat16,
                         kind="ExternalOutput")

    # 4-row-packed views: one view row = 4 consecutive 1KB rows = 4KB.
    # xq row c*128+p block j = xT row c*512 + 4p + j; Wq mirrors it, so the
    # matmul contraction (permutation-invariant over k) stays correct.
    xq = xT[:].rearrange("(r q) b -> r (q b)", q=Q)              # [1024, 2048]
    Wq = Wc[:].rearrange("a (r q) c -> (a r) (q c)", q=Q)        # [1024, 2048]

    with TileContext(nc) as tc:
        with tc.tile_pool(name="consts", bufs=1) as consts, \
             tc.tile_pool(name="route", bufs=1) as route, \
             tc.tile_pool(name="bfinp", bufs=1) as bfinp, \
             tc.tile_pool(name="hts", bufs=1) as hts, \
             tc.tile_pool(name="dram", bufs=1, space="DRAM") as dram:

            # ---------------- constants ----------------
            ones_col = consts.tile([P, 1], dt.float32)
            nc.vector.memset(ones_col[:], 1.0)
            ones_k1 = consts.tile([1, P], dt.float32)
            nc.vector.memset(ones_k1[:], 1.0)
            ones_k1h = consts.tile([1, P], dt.bfloat16)
            nc.vector.memset(ones_k1h[:], 1.0)
            ident = consts.tile([P, P], dt.float32)
            make_identity(nc, ident)
            # C8[p, c] = c  (chunk-id iota along free dim)
            C8 = consts.tile([P, NUM_CHUNKS], dt.int32)
            nc.gpsimd.iota(C8[:], pattern=[[1, NUM_CHUNKS]], base=0,
                           channel_multiplier=0)
            C8f = consts.tile([P, NUM_CHUNKS], dt.float32)
            nc.vector.tensor_copy(C8f[:], C8[:])

            hT = [[hts.tile([P, BS], dt.bfloat16, tag=f"ht{c}_{d}",
                            name=f"ht{c}_{d}") for d in range(DT_)]
                  for c in range(NUM_CHUNKS)]

            cc_in = dram.tile([P, NUM_CHUNKS], dt.float32)
            cc_out = dram.tile([P, NUM_CHUNKS], dt.float32)

            engs = [nc.sync, nc.scalar, nc.gpsimd]

            with tc.spectator_scope("pre"):
                with tc.tile_pool(name="xw", bufs=1) as xw, \
                     tc.tile_pool(name="ps_pre", bufs=2, space="PSUM") as ps_pre:
                    # tiny bias loads first (enable early PE prep work)
                    b_sb = route.tile([NUM_CHUNKS, COUT], dt.float32)
                    nc.sync.dma_start(b_sb[:], bc_t[:])
                    bfin = bfinp.tile([1, OUT_F], dt.float32)
                    nc.sync.dma_start(bfin[:], bf[:])

                    # ---- x loads: 3 rings, x gates the AR trigger ----
                    xs = []
                    for c in range(NUM_CHUNKS):
                        t = xw.tile([P, Q * BS], dt.bfloat16, tag=f"x{c}",
                                    name=f"x{c}")
                        engs[c % 3].dma_start(t[:], xq[c * P:(c + 1) * P, :])
                        xs.append(t)
                    # ---- Wc loads behind x on sync/scalar rings ----
                    ws = []
                    for c in range(NUM_CHUNKS):
                        t = xw.tile([P, Q * COUT], dt.bfloat16, tag=f"w{c}",
                                    name=f"w{c}")
                        engs[c % 2].dma_start(t[:], Wq[c * P:(c + 1) * P, :])
                        ws.append(t)

                    # ---- activities: per-chunk per-partition |x| sums ----
                    actcol = route.tile([P, NUM_CHUNKS], dt.float32)
                    for c in range(NUM_CHUNKS):
                        nc.vector.tensor_reduce(
                            actcol[:, c:c + 1], xs[c][:],
                            axis=mybir.AxisListType.X, op=mybir.AluOpType.add,
                            apply_absolute_value=True)
                    # AR input is the whole [128, 8] tile; partition reduce
                    # happens post-AR so the PE stays free for L1 here.
                    nc.gpsimd.dma_start(cc_in[:], actcol[:])
                    nc.gpsimd.collective_compute(
                        "AllReduce", mybir.AluOpType.add,
                        replica_groups=[list(range(N_CORES))],
                        ins=[cc_in.opt()], outs=[cc_out.opt()])

                    # ---- bias prep (PE transposes + bfin broadcast) ----
                    bT = route.tile([P, DT_ * NUM_CHUNKS], dt.float32)
                    for d in range(DT_):
                        ps_t = ps_pre.tile([P, NUM_CHUNKS], dt.float32,
                                           tag="pst")
                        nc.tensor.transpose(
                            ps_t[:], b_sb[:, d * P:(d + 1) * P],
                            ident[0:NUM_CHUNKS, 0:NUM_CHUNKS])
                        nc.scalar.copy(
                            bT[:, d * NUM_CHUNKS:(d + 1) * NUM_CHUNKS],
                            ps_t[:])
                    bfin_h = bfinp.tile([1, OUT_F], dt.bfloat16)
                    nc.vector.tensor_copy(bfin_h[:], bfin[:])
                    bfin_bc = bfinp.tile([P, OUT_F], dt.float32)
                    for o in range(OT):
                        sl = slice(o * 512, (o + 1) * 512)
                        ps_b = ps_pre.tile([P, 512], dt.float32, tag="psb")
                        nc.tensor.matmul(ps_b[:], ones_k1h[:], bfin_h[:, sl],
                                         start=True, stop=True)
                        nc.vector.tensor_copy(bfin_bc[:, sl], ps_b[:])

                    # ---- L1 for ALL 8 chunks (runs inside the AR window) --
                    with tc.tile_pool(name="ps_h", bufs=4,
                                      space="PSUM") as ps_h:
                        for c in range(NUM_CHUNKS):
                            for d in range(DT_):
                                ph = ps_h.tile([P, BS], dt.float32, tag="ph",
                                               name=f"ph{c}_{d}")
                                for j in range(Q):
                                    nc.tensor.matmul(
                                        ph[:],
                                        ws[c][:, j * COUT + d * P:
                                              j * COUT + (d + 1) * P],
                                        xs[c][:, j * BS:(j + 1) * BS],
                                        start=(j == 0), stop=(j == Q - 1))
                                nc.scalar.activation(
                                    hT[c][d][:], ph[:],
                                    mybir.ActivationFunctionType.Identity,
                                    bias=bT[:, d * NUM_CHUNKS + c:
                                            d * NUM_CHUNKS + c + 1])

            # ---------------- routing (post-AR) ----------------
            with tc.spectator_scope("route"):
                with tc.tile_pool(name="ps_rt", bufs=1,
                                  space="PSUM") as ps_rt:
                    act_g = route.tile([P, NUM_CHUNKS], dt.float32)
                    nc.gpsimd.dma_start(act_g[:], cc_out[:])
                    act_ps = ps_rt.tile([1, NUM_CHUNKS], dt.float32, tag="psa")
                    nc.tensor.matmul(act_ps[:], ones_col[:], act_g[:],
                                     start=True, stop=True)
                    act_row = route.tile([1, NUM_CHUNKS], dt.float32)
                    nc.scalar.copy(act_row[:], act_ps[:])

                    maxv = route.tile([1, NUM_CHUNKS], dt.float32)
                    maxi = route.tile([1, NUM_CHUNKS], dt.uint32)
                    nc.vector.max(maxv[:], act_row[:])
                    nc.vector.max_index(maxi[:], maxv[:], act_row[:])
                    maxi_f = route.tile([1, NUM_CHUNKS], dt.float32)
                    nc.vector.tensor_copy(maxi_f[:], maxi[:])

                    # bcast[p, j] = idx[j] on every partition (K=1 matmul)
                    bc_ps = ps_rt.tile([P, NUM_CHUNKS], dt.float32, tag="psc")
                    nc.tensor.matmul(bc_ps[:], ones_k1[:], maxi_f[:],
                                     start=True, stop=True)
                    bcast = route.tile([P, NUM_CHUNKS], dt.float32)
                    nc.vector.tensor_copy(bcast[:], bc_ps[:])

                    # gates[p, s*8+c] = (c == sel_s), in bf16 (exact 0/1)
                    onehot = route.tile([P, TOP_K * NUM_CHUNKS], dt.float32)
                    for s in range(TOP_K):
                        nc.vector.tensor_scalar(
                            onehot[:, s * NUM_CHUNKS:(s + 1) * NUM_CHUNKS],
                            C8f[:], bcast[:, s:s + 1], scalar2=None,
                            op0=mybir.AluOpType.is_equal)


                    # ---- PE warmup during route/select (act_g-gated) ----
                    warm_rhs = route.tile([1, 256], dt.bfloat16)
                    nc.vector.tensor_scalar(
                        warm_rhs[:], bfin_h[0:1, 0:256],
                        act_row[0:1, 0:1], scalar2=None,
                        op0=mybir.AluOpType.add)
                    for wi in range(10):
                        ps_w = ps_rt.tile([P, 256], dt.float32, tag="psw")
                        nc.tensor.matmul(ps_w[:], ones_k1h[:], warm_rhs[:],
                                         start=True, stop=True)

            # ---------------- select + L2 ----------------
            with tc.spectator_scope("l2"):
                with tc.tile_pool(name="wfs", bufs=1) as wfs, \
                     tc.tile_pool(name="hsel_p", bufs=1) as hsel_p, \
                     tc.tile_pool(name="outs", bufs=1) as outs, \
                     tc.tile_pool(name="ps_o", bufs=8, space="PSUM") as ps_o:
                    # FIFO blockers: keep the W_final slabs off the rings
                    # until the collective completes.
                    blk0 = route.tile([1, NUM_CHUNKS], dt.float32)
                    blk1 = route.tile([1, NUM_CHUNKS], dt.float32)
                    nc.sync.dma_start(blk0[:], cc_out[0:1, :])
                    nc.scalar.dma_start(blk1[:], cc_out[0:1, :])
                    wfo = []
                    for o in range(OT):
                        t = wfs.tile([P, OUT_F], dt.bfloat16, tag=f"wf{o}",
                                     name=f"wf{o}")
                        engs[o % 2].dma_start(t[:], Wfr[o * P:(o + 1) * P, :])
                        wfo.append(t)

                    # ---- select: hsel[kf] = sum_c gate[s][c] * hT[c][d] ----
                    hsel = []
                    for kf in range(KF):
                        s, d = divmod(kf, DT_)
                        t = hsel_p.tile([P, BS], dt.bfloat16, tag=f"hs{kf}",
                                        name=f"hs{kf}")
                        nc.vector.tensor_scalar(
                            t[:], hT[0][d][:],
                            onehot[:, s * NUM_CHUNKS:s * NUM_CHUNKS + 1],
                            scalar2=None, op0=mybir.AluOpType.mult)
                        for c in range(1, NUM_CHUNKS):
                            nc.vector.scalar_tensor_tensor(
                                out=t[:], in0=hT[c][d][:],
                                scalar=onehot[:, s * NUM_CHUNKS + c:
                                              s * NUM_CHUNKS + c + 1],
                                in1=t[:], op0=mybir.AluOpType.mult,
                                op1=mybir.AluOpType.add)
                        hsel.append(t)

                    # ---- L2: o-outer so wfo streams; out in half-rows ----
                    orow = [outs.tile([P, OUT_F], dt.bfloat16, tag=f"or{bt}",
                                      name=f"or{bt}") for bt in range(BT)]
                    for o in range(OT):
                        osl = slice(o * 512, (o + 1) * 512)
                        po = [ps_o.tile([P, 512], dt.float32, tag="po",
                                        name=f"po{o}_{bt}")
                              for bt in range(BT)]
                        for kf in range(KF):
                            for bt in range(BT):
                                nc.tensor.matmul(
                                    po[bt][:],
                                    hsel[kf][:, bt * P:(bt + 1) * P],
                                    wfo[o][:, kf * 512:(kf + 1) * 512],
                                    start=(kf == 0), stop=(kf == KF - 1))
                        for bt in range(BT):
                            nc.vector.tensor_tensor(
                                out=orow[bt][:, osl], in0=po[bt][:],
                                in1=bfin_bc[:, osl], op=mybir.AluOpType.add)
                        if o == OT // 2 - 1:
                            for bt in range(BT):
                                engs[bt % 2].dma_start(
                                    out[bt * P:(bt + 1) * P, 0:OUT_F // 2],
                                    orow[bt][:, 0:OUT_F // 2])
                    for bt in range(BT):
                        engs[bt % 2].dma_start(
                            out[bt * P:(bt + 1) * P, OUT_F // 2:OUT_F],
                            orow[bt][:, OUT_F // 2:OUT_F])
    nc.compile()
    return nc


def kernel(x, W_chunks, b_chunks, W_final, b_final):
    bf16 = ml_dtypes.bfloat16
    x = np.asarray(x, dtype=np.float32).astype(bf16)
    W_chunks = np.asarray(W_chunks, dtype=np.float32).astype(bf16)
    W_final = np.asarray(W_final, dtype=np.float32).astype(bf16)
    b_chunks = np.ascontiguousarray(np.asarray(b_chunks, dtype=np.float32))
    b_final = np.ascontiguousarray(
        np.asarray(b_final, dtype=np.float32).reshape(1, OUT_F))

    # o-major repack of W_final: Wfr[o*128+p, kf*512+n] = Wf[kf*128+p, o*512+n]
    Wfr = np.ascontiguousarray(
        W_final.reshape(KF, P, OT, 512).transpose(2, 1, 0, 3)
        .reshape(OT * P, KF * 512))

    if "nc" not in _cache:
        _cache["nc"] = _build()
    nc = _cache["nc"]

    in_maps = [{
        "xT_shard": np.ascontiguousarray(x[c * BS:(c + 1) * BS].T),
        "W_chunks": W_chunks,
        "b_chunks": b_chunks,
        "W_final_r": Wfr,
        "b_final": b_final,
    } for c in range(N_CORES)]

    res = run_bass_kernel_spmd(nc, in_maps, core_ids=list(range(N_CORES)))
    kernel.last_result = res
    return np.concatenate(
        [res.results[c]["out_shard"].astype(np.float32)
         for c in range(N_CORES)], axis=0)


kernel.last_result = None


# revision 8
# speedup vs baseline: 1.1459x; 1.1459x over previous
"""Trainium2 Bass kernel for BlockChunkedActivityRoutedNet (v3).

Reference computation (B=4096, IN_F=4096, 8 chunks of 512, top-2 by mean|x|,
chunk-expert Linears 512->512, concat -> final Linear 1024->4096):

    xr = x.reshape(B, 8, 512)
    activities = mean(|xr|, axis=(0, 2))            # over the WHOLE batch
    i0, i1 = top2(activities)                        # descending
    h = concat(xr[:, i0] @ Wc[i0] + bc[i0], xr[:, i1] @ Wc[i1] + bc[i1])
    out = h @ W_final + b_final

Distribution: data-parallel over the batch across 8 NeuronCores (512 rows
each).  Per-chunk per-partition |x| partial sums are AllReduced as a [128, 8]
f32 tile; partition reduce + top-2 run post-AR identically on every core.

v3 design (evolved from the v1/v2 traces):
  - L1 runs SPECULATIVELY for ALL 8 chunks during the collective's entry
    barrier + AR machinery window (the PE is idle there anyway; measured
    window = launch skew + ~25us of ncfw stepping).
  - hT[c] (bias added, bf16, [128, 2048] d-major) is written to an internal
    DRAM tensor over the otherwise-idle SWDGE ring as each chunk finishes —
    well before the AR data phase, so it does not inflate the collective.
  - Post-AR the routing picks top-2; TWO indirect gathers ([128, 2048],
    4KB rows) pull the selected chunks' hT back into SBUF.  This replaces
    v2's DVE gated-sum select (measured 745ns/op -> 39us serial chain that
    paced the whole L2 start).
  - Activity reduces split DVE/GpSimd (v2 ran all 8 on DVE: 2.3us each,
    18us serial chain on the AR-trigger path).
  - W_final host-repacked o-major; L2 loops o-outer so the 8MB streams in
    just ahead of consumption; wfo DMAs carry explicit deps on the
    cc_out-read FIFO blockers (v2 leaked 3 tiles into the AR window).
  - out streams in quarter-rows after o=1,3,5,7 (kills the out tail).
"""

import numpy as np
import ml_dtypes

import concourse.bass as bass
import concourse.bacc as bacc
import concourse.mybir as mybir
from concourse.tile import TileContext, add_dep_helper
from concourse.bass_utils import run_bass_kernel_spmd
from concourse.masks import make_identity

dt = mybir.dt
P = 128

NUM_CHUNKS = 8
TOP_K = 2
IN_F = 4096
HID_F = 4096
OUT_F = 4096
B = 4096
CIN = IN_F // NUM_CHUNKS      # 512
COUT = HID_F // NUM_CHUNKS    # 512
N_CORES = 8
BS = B // N_CORES             # 512 rows per core

BT = BS // P                  # 4 batch tiles per core
DT_ = COUT // P               # 4 d-tiles per chunk
KF = TOP_K * DT_              # 8 k-tiles for the final matmul
OT = OUT_F // 512             # 8 output column tiles of 512
Q = 4                         # DRAM row packing for x / Wc views

_cache = {}


def _build():
    nc = bacc.Bacc(num_devices=N_CORES, name="chunk_routed_v3",
                   num_swdge_queues=4)

    xT = nc.dram_tensor("xT_shard", [IN_F, BS], dt.bfloat16,
                        kind="ExternalInput")
    Wc = nc.dram_tensor("W_chunks", [NUM_CHUNKS, CIN, COUT], dt.bfloat16,
                        kind="ExternalInput")
    bc_t = nc.dram_tensor("b_chunks", [NUM_CHUNKS, COUT], dt.float32,
                          kind="ExternalInput")
    Wfr = nc.dram_tensor("W_final_r", [KF * P, OUT_F], dt.bfloat16,
                         kind="ExternalInput")
    bf = nc.dram_tensor("b_final", [1, OUT_F], dt.float32, kind="ExternalInput")
    out = nc.dram_tensor("out_shard", [BS, OUT_F], dt.bfloat16,
                         kind="ExternalOutput")

    # 4-row-packed views: one view row = 4 consecutive 1KB rows = 4KB.
    # xq row c*128+p block j = xT row c*512 + 4p + j; Wq mirrors it, so the
    # matmul contraction (permutation-invariant over k) stays correct.
    xq = xT[:].rearrange("(r q) b -> r (q b)", q=Q)              # [1024, 2048]
    Wq = Wc[:].rearrange("a (r q) c -> (a r) (q c)", q=Q)        # [1024, 2048]

    with TileContext(nc) as tc:
        with tc.tile_pool(name="consts", bufs=1) as consts, \
             tc.tile_pool(name="route", bufs=1) as route, \
             tc.tile_pool(name="bfinp", bufs=1) as bfinp, \
             tc.tile_pool(name="hts", bufs=1) as hts, \
             tc.tile_pool(name="dram", bufs=1, space="DRAM") as dram:

            # ---------------- constants ----------------
            ones_col = consts.tile([P, 1], dt.float32)
            nc.vector.memset(ones_col[:], 1.0)
            ones_k1 = consts.tile([1, P], dt.float32)
            nc.vector.memset(ones_k1[:], 1.0)
            ones_k1h = consts.tile([1, P], dt.bfloat16)
            nc.vector.memset(ones_k1h[:], 1.0)
            ident = consts.tile([P, P], dt.float32)
            make_identity(nc, ident)
            # C_P[p, 0] = p  (row offset within the hT gather view)
            C_P = consts.tile([P, 1], dt.int32)
            nc.gpsimd.iota(C_P[:], pattern=[[P, 1]], base=0,
                           channel_multiplier=1)
            C_Pf = consts.tile([P, 1], dt.float32)
            nc.vector.tensor_copy(C_Pf[:], C_P[:])

            # hT[c]: [128, d*512 + b] (bias-added L1 output, bf16)
            hT = [hts.tile([P, DT_ * BS], dt.bfloat16, tag=f"ht{c}",
                           name=f"ht{c}") for c in range(NUM_CHUNKS)]

            cc_in = dram.tile([P, NUM_CHUNKS], dt.float32)
            cc_out = dram.tile([P, NUM_CHUNKS], dt.float32)
            hT_d = dram.tile([NUM_CHUNKS * P, DT_ * BS], dt.bfloat16)

            engs = [nc.sync, nc.scalar, nc.gpsimd]

            with tc.spectator_scope("pre"):
                with tc.tile_pool(name="xw", bufs=1) as xw, \
                     tc.tile_pool(name="ps_pre", bufs=2, space="PSUM") as ps_pre:
                    # tiny bias loads first (enable early PE prep work)
                    b_sb = route.tile([NUM_CHUNKS, COUT], dt.float32)
                    nc.sync.dma_start(b_sb[:], bc_t[:])
                    bfin = bfinp.tile([1, OUT_F], dt.float32)
                    nc.sync.dma_start(bfin[:], bf[:])

                    # ---- x loads: 3 rings, x gates the AR trigger ----
                    xs = []
                    for c in range(NUM_CHUNKS):
                        t = xw.tile([P, Q * BS], dt.bfloat16, tag=f"x{c}",
                                    name=f"x{c}")
                        engs[c % 3].dma_start(t[:], xq[c * P:(c + 1) * P, :])
                        xs.append(t)
                    # ---- Wc loads behind x on sync/scalar rings ----
                    ws = []
                    for c in range(NUM_CHUNKS):
                        t = xw.tile([P, Q * COUT], dt.bfloat16, tag=f"w{c}",
                                    name=f"w{c}")
                        engs[c % 2].dma_start(t[:], Wq[c * P:(c + 1) * P, :])
                        ws.append(t)

                    # ---- activities: per-chunk per-partition |x| sums ----
                    # Split DVE/ACT: a single-engine chain of 8 reduces
                    # (2.3us each) would gate the AR trigger.  Odd chunks run
                    # on the scalar engine as Abs-activation with accum_out
                    # (per-partition row sum); the ACT is otherwise idle
                    # until the L1 evictions start.
                    actcol = route.tile([P, NUM_CHUNKS], dt.float32)
                    abs_scr = xw.tile([P, Q * BS], dt.bfloat16, tag="abs_scr")
                    for c in range(NUM_CHUNKS):
                        if c % 2 == 0:
                            nc.vector.tensor_reduce(
                                actcol[:, c:c + 1], xs[c][:],
                                axis=mybir.AxisListType.X,
                                op=mybir.AluOpType.add,
                                apply_absolute_value=True)
                        else:
                            nc.scalar.activation(
                                abs_scr[:], xs[c][:],
                                mybir.ActivationFunctionType.Abs,
                                accum_out=actcol[:, c:c + 1])
                    # AR input is the whole [128, 8] tile; partition reduce
                    # happens post-AR so the PE stays free for L1 here.
                    nc.gpsimd.dma_start(cc_in[:], actcol[:])
                    nc.gpsimd.collective_compute(
                        "AllReduce", mybir.AluOpType.add,
                        replica_groups=[list(range(N_CORES))],
                        ins=[cc_in.opt()], outs=[cc_out.opt()])

                    # ---- bias prep (PE transposes + bfin broadcast) ----
                    bT = route.tile([P, DT_ * NUM_CHUNKS], dt.float32)
                    for d in range(DT_):
                        ps_t = ps_pre.tile([P, NUM_CHUNKS], dt.float32,
                                           tag="pst")
                        nc.tensor.transpose(
                            ps_t[:], b_sb[:, d * P:(d + 1) * P],
                            ident[0:NUM_CHUNKS, 0:NUM_CHUNKS])
                        nc.scalar.copy(
                            bT[:, d * NUM_CHUNKS:(d + 1) * NUM_CHUNKS],
                            ps_t[:])
                    bfin_h = bfinp.tile([1, OUT_F], dt.bfloat16)
                    nc.vector.tensor_copy(bfin_h[:], bfin[:])
                    bfin_bc = bfinp.tile([P, OUT_F], dt.float32)
                    for o in range(OT):
                        sl = slice(o * 512, (o + 1) * 512)
                        ps_b = ps_pre.tile([P, 512], dt.float32, tag="psb")
                        nc.tensor.matmul(ps_b[:], ones_k1h[:], bfin_h[:, sl],
                                         start=True, stop=True)
                        nc.vector.tensor_copy(bfin_bc[:, sl], ps_b[:])

                    # ---- L1 for ALL 8 chunks (runs inside the AR window);
                    #      each finished chunk streams to DRAM over SWDGE --
                    with tc.tile_pool(name="ps_h", bufs=4,
                                      space="PSUM") as ps_h:
                        for c in range(NUM_CHUNKS):
                            for d in range(DT_):
                                ph = ps_h.tile([P, BS], dt.float32, tag="ph",
                                               name=f"ph{c}_{d}")
                                for j in range(Q):
                                    nc.tensor.matmul(
                                        ph[:],
                                        ws[c][:, j * COUT + d * P:
                                              j * COUT + (d + 1) * P],
                                        xs[c][:, j * BS:(j + 1) * BS],
                                        start=(j == 0), stop=(j == Q - 1))
                                nc.scalar.activation(
                                    hT[c][:, d * BS:(d + 1) * BS], ph[:],
                                    mybir.ActivationFunctionType.Identity,
                                    bias=bT[:, d * NUM_CHUNKS + c:
                                            d * NUM_CHUNKS + c + 1])
                            nc.gpsimd.dma_start(
                                hT_d[c * P:(c + 1) * P, :], hT[c][:])

            # ---------------- routing (post-AR) ----------------
            with tc.spectator_scope("route"):
                with tc.tile_pool(name="ps_rt", bufs=1,
                                  space="PSUM") as ps_rt:
                    act_g = route.tile([P, NUM_CHUNKS], dt.float32)
                    nc.gpsimd.dma_start(act_g[:], cc_out[:])
                    act_ps = ps_rt.tile([1, NUM_CHUNKS], dt.float32, tag="psa")
                    nc.tensor.matmul(act_ps[:], ones_col[:], act_g[:],
                                     start=True, stop=True)
                    act_row = route.tile([1, NUM_CHUNKS], dt.float32)
                    nc.scalar.copy(act_row[:], act_ps[:])

                    maxv = route.tile([1, NUM_CHUNKS], dt.float32)
                    maxi = route.tile([1, NUM_CHUNKS], dt.uint32)
                    nc.vector.max(maxv[:], act_row[:])
                    nc.vector.max_index(maxi[:], maxv[:], act_row[:])
                    maxi_f = route.tile([1, NUM_CHUNKS], dt.float32)
                    nc.vector.tensor_copy(maxi_f[:], maxi[:])

                    # bcast[p, j] = idx[j] on every partition (K=1 matmul)
                    bc_ps = ps_rt.tile([P, NUM_CHUNKS], dt.float32, tag="psc")
                    nc.tensor.matmul(bc_ps[:], ones_k1[:], maxi_f[:],
                                     start=True, stop=True)
                    bcast = route.tile([P, NUM_CHUNKS], dt.float32)
                    nc.vector.tensor_copy(bcast[:], bc_ps[:])

                    # gather offsets: off[p, s] = sel_s*128 + p
                    bc128 = route.tile([P, TOP_K], dt.float32)
                    nc.vector.tensor_scalar_mul(bc128[:], bcast[:, 0:TOP_K],
                                                128.0)
                    off_f = route.tile([P, TOP_K], dt.float32)
                    for s in range(TOP_K):
                        nc.vector.tensor_scalar(
                            off_f[:, s:s + 1], C_Pf[:], bc128[:, s:s + 1],
                            scalar2=None, op0=mybir.AluOpType.add)
                    off = route.tile([P, TOP_K], dt.int32)
                    nc.vector.tensor_copy(off[:], off_f[:])

                    # ---- PE warmup during route/gather (act_g-gated) ----
                    warm_rhs = route.tile([1, 256], dt.bfloat16)
                    nc.vector.tensor_scalar(
                        warm_rhs[:], bfin_h[0:1, 0:256],
                        act_row[0:1, 0:1], scalar2=None,
                        op0=mybir.AluOpType.add)
                    for wi in range(10):
                        ps_w = ps_rt.tile([P, 256], dt.float32, tag="psw")
                        nc.tensor.matmul(ps_w[:], ones_k1h[:], warm_rhs[:],
                                         start=True, stop=True)

            # ---------------- gather selected hT + L2 ----------------
            with tc.spectator_scope("l2"):
                with tc.tile_pool(name="wfs", bufs=1) as wfs, \
                     tc.tile_pool(name="hsel_p", bufs=1) as hsel_p, \
                     tc.tile_pool(name="outs", bufs=1) as outs, \
                     tc.tile_pool(name="ps_o", bufs=8, space="PSUM") as ps_o:
                    # hsel[s][p, d*512+b] = hT[sel_s][p, d*512+b]
                    hsel = [hsel_p.tile([P, DT_ * BS], dt.bfloat16,
                                        tag=f"hs{s}", name=f"hs{s}")
                            for s in range(TOP_K)]
                    for s in range(TOP_K):
                        nc.gpsimd.indirect_dma_start(
                            out=hsel[s][:], out_offset=None,
                            in_=hT_d[:],
                            in_offset=bass.IndirectOffsetOnAxis(
                                ap=off[:, s:s + 1], axis=0))

                    # FIFO blockers: keep the W_final slabs off the rings
                    # until the collective completes (explicit deps — the
                    # scheduler otherwise hoists some wfo loads into the AR
                    # window, inflating the collective).
                    blk0 = route.tile([1, NUM_CHUNKS], dt.float32)
                    blk1 = route.tile([1, NUM_CHUNKS], dt.float32)
                    bi0 = nc.sync.dma_start(blk0[:], cc_out[0:1, :])
                    bi1 = nc.scalar.dma_start(blk1[:], cc_out[0:1, :])
                    wfo = []
                    for o in range(OT):
                        t = wfs.tile([P, OUT_F], dt.bfloat16, tag=f"wf{o}",
                                     name=f"wf{o}")
                        di = engs[o % 2].dma_start(
                            t[:], Wfr[o * P:(o + 1) * P, :])
                        add_dep_helper(di.ins, (bi0 if o % 2 == 0 else bi1).ins,
                                       sync=False,
                                       reason="wfo loads after AR blocker")
                        wfo.append(t)

                    # ---- L2: o-outer so wfo streams; out in quarter-rows --
                    orow = [outs.tile([P, OUT_F], dt.bfloat16, tag=f"or{bt}",
                                      name=f"or{bt}") for bt in range(BT)]
                    for o in range(OT):
                        osl = slice(o * 512, (o + 1) * 512)
                        po = [ps_o.tile([P, 512], dt.float32, tag="po",
                                        name=f"po{o}_{bt}")
                              for bt in range(BT)]
                        for kf in range(KF):
                            s, d = divmod(kf, DT_)
                            for bt in range(BT):
                                nc.tensor.matmul(
                                    po[bt][:],
                                    hsel[s][:, d * BS + bt * P:
                                            d * BS + (bt + 1) * P],
                                    wfo[o][:, kf * 512:(kf + 1) * 512],
                                    start=(kf == 0), stop=(kf == KF - 1))
                        for bt in range(BT):
                            nc.vector.tensor_tensor(
                                out=orow[bt][:, osl], in0=po[bt][:],
                                in1=bfin_bc[:, osl], op=mybir.AluOpType.add)
                        if o % 2 == 1:
                            qsl = slice((o - 1) * 512, (o + 1) * 512)
                            for bt in range(BT):
                                engs[bt % 2].dma_start(
                                    out[bt * P:(bt + 1) * P, qsl],
                                    orow[bt][:, qsl])
    nc.compile()
    return nc


def kernel(x, W_chunks, b_chunks, W_final, b_final):
    bf16 = ml_dtypes.bfloat16
    x = np.asarray(x, dtype=np.float32).astype(bf16)
    W_chunks = np.asarray(W_chunks, dtype=np.float32).astype(bf16)
    W_final = np.asarray(W_final, dtype=np.float32).astype(bf16)
    b_chunks = np.ascontiguousarray(np.asarray(b_chunks, dtype=np.float32))
    b_final = np.ascontiguousarray(
        np.asarray(b_final, dtype=np.float32).reshape(1, OUT_F))

    # o-major repack of W_final: Wfr[o*128+p, kf*512+n] = Wf[kf*128+p, o*512+n]
    Wfr = np.ascontiguousarray(
        W_final.reshape(KF, P, OT, 512).transpose(2, 1, 0, 3)
        .reshape(OT * P, KF * 512))

    if "nc" not in _cache:
        _cache["nc"] = _build()
    nc = _cache["nc"]

    in_maps = [{
        "xT_shard": np.ascontiguousarray(x[c * BS:(c + 1) * BS].T),
        "W_chunks": W_chunks,
        "b_chunks": b_chunks,
        "W_final_r": Wfr,
        "b_final": b_final,
    } for c in range(N_CORES)]

    res = run_bass_kernel_spmd(nc, in_maps, core_ids=list(range(N_CORES)))
    kernel.last_result = res
    return np.concatenate(
        [res.results[c]["out_shard"].astype(np.float32)
         for c in range(N_CORES)], axis=0)


kernel.last_result = None


# revision 16
# speedup vs baseline: 1.2168x; 1.0619x over previous
"""Trainium2 Bass kernel for BlockChunkedActivityRoutedNet (v3).

Reference computation (B=4096, IN_F=4096, 8 chunks of 512, top-2 by mean|x|,
chunk-expert Linears 512->512, concat -> final Linear 1024->4096):

    xr = x.reshape(B, 8, 512)
    activities = mean(|xr|, axis=(0, 2))            # over the WHOLE batch
    i0, i1 = top2(activities)                        # descending
    h = concat(xr[:, i0] @ Wc[i0] + bc[i0], xr[:, i1] @ Wc[i1] + bc[i1])
    out = h @ W_final + b_final

Distribution: data-parallel over the batch across 8 NeuronCores (512 rows
each).  Per-chunk per-partition |x| partial sums are AllReduced as a [128, 8]
f32 tile; partition reduce + top-2 run post-AR identically on every core.

v3 design (evolved from the v1/v2 traces):
  - L1 runs SPECULATIVELY for ALL 8 chunks during the collective's entry
    barrier + AR machinery window (the PE is idle there anyway; measured
    window = launch skew + ~25us of ncfw stepping).
  - hT[c] (bias added, bf16, [128, 2048] d-major) is written to an internal
    DRAM tensor over the otherwise-idle SWDGE ring as each chunk finishes —
    well before the AR data phase, so it does not inflate the collective.
  - Post-AR the routing picks top-2; TWO indirect gathers ([128, 2048],
    4KB rows) pull the selected chunks' hT back into SBUF.  This replaces
    v2's DVE gated-sum select (measured 745ns/op -> 39us serial chain that
    paced the whole L2 start).
  - Activity reduces split DVE/GpSimd (v2 ran all 8 on DVE: 2.3us each,
    18us serial chain on the AR-trigger path).
  - W_final host-repacked o-major; L2 loops o-outer so the 8MB streams in
    just ahead of consumption; wfo DMAs carry explicit deps on the
    cc_out-read FIFO blockers (v2 leaked 3 tiles into the AR window).
  - out streams in quarter-rows after o=1,3,5,7 (kills the out tail).
"""

import numpy as np
import ml_dtypes

import concourse.bass as bass
import concourse.bacc as bacc
import concourse.mybir as mybir
from concourse.tile import TileContext, add_dep_helper
from concourse.bass_utils import run_bass_kernel_spmd
from concourse.masks import make_identity

dt = mybir.dt
P = 128

NUM_CHUNKS = 8
TOP_K = 2
IN_F = 4096
HID_F = 4096
OUT_F = 4096
B = 4096
CIN = IN_F // NUM_CHUNKS      # 512
COUT = HID_F // NUM_CHUNKS    # 512
N_CORES = 8
BS = B // N_CORES             # 512 rows per core

BT = BS // P                  # 4 batch tiles per core
DT_ = COUT // P               # 4 d-tiles per chunk
KF = TOP_K * DT_              # 8 k-tiles for the final matmul
OT = OUT_F // 512             # 8 output column tiles of 512
Q = 4                         # DRAM row packing for x / Wc views

_cache = {}


def _build():
    nc = bacc.Bacc(num_devices=N_CORES, name="chunk_routed_v3",
                   num_swdge_queues=4)

    xT = nc.dram_tensor("xT_shard", [IN_F, BS], dt.bfloat16,
                        kind="ExternalInput")
    Wc = nc.dram_tensor("W_chunks", [NUM_CHUNKS, CIN, COUT], dt.bfloat16,
                        kind="ExternalInput")
    bc_t = nc.dram_tensor("b_chunks", [NUM_CHUNKS, COUT], dt.float32,
                          kind="ExternalInput")
    Wfr = nc.dram_tensor("W_final_r", [KF * P, OUT_F], dt.bfloat16,
                         kind="ExternalInput")
    bf = nc.dram_tensor("b_final", [1, OUT_F], dt.float32, kind="ExternalInput")
    out = nc.dram_tensor("out_shard", [BS, OUT_F], dt.bfloat16,
                         kind="ExternalOutput")

    # 4-row-packed views: one view row = 4 consecutive 1KB rows = 4KB.
    # xq row c*128+p block j = xT row c*512 + 4p + j; Wq mirrors it, so the
    # matmul contraction (permutation-invariant over k) stays correct.
    xq = xT[:].rearrange("(r q) b -> r (q b)", q=Q)              # [1024, 2048]
    Wq = Wc[:].rearrange("a (r q) c -> (a r) (q c)", q=Q)        # [1024, 2048]

    with TileContext(nc) as tc:
        with tc.tile_pool(name="consts", bufs=1) as consts, \
             tc.tile_pool(name="route", bufs=1) as route, \
             tc.tile_pool(name="bfinp", bufs=1) as bfinp, \
             tc.tile_pool(name="hts", bufs=1) as hts, \
             tc.tile_pool(name="dram", bufs=1, space="DRAM") as dram:

            # ---------------- constants ----------------
            ones_col = consts.tile([P, 1], dt.float32)
            nc.vector.memset(ones_col[:], 1.0)
            ones_k1 = consts.tile([1, P], dt.float32)
            nc.vector.memset(ones_k1[:], 1.0)
            ones_k1h = consts.tile([1, P], dt.bfloat16)
            nc.vector.memset(ones_k1h[:], 1.0)
            ident = consts.tile([P, P], dt.float32)
            make_identity(nc, ident)
            # C_P[p, 0] = p  (row offset within the hT gather view)
            C_P = consts.tile([P, 1], dt.int32)
            nc.gpsimd.iota(C_P[:], pattern=[[P, 1]], base=0,
                           channel_multiplier=1)
            C_Pf = consts.tile([P, 1], dt.float32)
            nc.vector.tensor_copy(C_Pf[:], C_P[:])

            # hT[c]: [128, d*512 + b] (bias-added L1 output, bf16)
            hT = [hts.tile([P, DT_ * BS], dt.bfloat16, tag=f"ht{c}",
                           name=f"ht{c}") for c in range(NUM_CHUNKS)]

            cc_in = dram.tile([1, 2 * NUM_CHUNKS], dt.float32)
            cc_out = dram.tile([1, 2 * NUM_CHUNKS], dt.float32)
            hT_d = dram.tile([NUM_CHUNKS * P, DT_ * BS], dt.bfloat16)

            engs = [nc.sync, nc.scalar, nc.gpsimd]

            with tc.spectator_scope("pre"):
                with tc.tile_pool(name="xw", bufs=1) as xw, \
                     tc.tile_pool(name="ps_pre", bufs=1, space="PSUM") as ps_pre, \
                     tc.tile_pool(name="ps_act", bufs=1, space="PSUM") as ps_act:
                    # tiny bias loads first (enable early PE prep work)
                    b_sb = route.tile([NUM_CHUNKS, COUT], dt.float32)
                    nc.sync.dma_start(b_sb[:], bc_t[:])
                    bfin = bfinp.tile([1, OUT_F], dt.float32)
                    nc.sync.dma_start(bfin[:], bf[:])

                    # ---- x loads: 16 half-chunk tiles over 3 rings so the
                    # activity reduces pipeline finely (x gates the trigger)
                    xs = [None] * NUM_CHUNKS
                    xh = []
                    for i in range(2 * NUM_CHUNKS):
                        c, h = divmod(i, 2)
                        if h == 0:
                            xs[c] = xw.tile([P, Q * BS], dt.bfloat16,
                                            tag=f"x{c}", name=f"x{c}")
                        t = xs[c][:, h * Q * BS // 2:(h + 1) * Q * BS // 2]
                        engs[i % 3].dma_start(
                            t, xq[c * P:(c + 1) * P,
                                  h * Q * BS // 2:(h + 1) * Q * BS // 2])
                        xh.append(t)

                    # ---- activities: per-half-chunk per-partition |x| sums
                    # split DVE/ACT (one engine alone would gate the trigger)
                    acth = route.tile([P, 16], dt.float32)
                    abs_scr = xw.tile([P, Q * BS // 2], dt.bfloat16,
                                      tag="abs_scr")
                    for i in range(2 * NUM_CHUNKS):
                        c, h = divmod(i, 2)
                        col = h * NUM_CHUNKS + c
                        if i % 2 == 0:
                            nc.vector.tensor_reduce(
                                acth[:, col:col + 1], xh[i],
                                axis=mybir.AxisListType.X,
                                op=mybir.AluOpType.add,
                                apply_absolute_value=True)
                        else:
                            nc.scalar.activation(
                                abs_scr[:], xh[i],
                                mybir.ActivationFunctionType.Abs,
                                accum_out=acth[:, col:col + 1])
                    # partition-reduce on PE: with Wc queued behind the
                    # trigger write, the PE sits idle here anyway, so this
                    # tiny matmul costs nothing on the L1 path.
                    act_ps = ps_act.tile([1, 16], dt.float32, tag="psa")
                    nc.tensor.matmul(act_ps[:], ones_col[:], acth[:],
                                     start=True, stop=True)
                    act_row = route.tile([1, 16], dt.float32)
                    nc.scalar.copy(act_row[:], act_ps[:])
                    # 64B trigger write on the sync HWDGE ring (single
                    # packet; the SWDGE [128,8] write's completion receipt
                    # cost ~6us on the trigger path in v3)
                    nc.sync.dma_start(cc_in[:], act_row[:])
                    nc.gpsimd.collective_compute(
                        "AllReduce", mybir.AluOpType.add,
                        replica_groups=[list(range(N_CORES))],
                        ins=[cc_in.opt()], outs=[cc_out.opt()])

                    # ---- Wc loads behind x (and behind the sync-ring
                    # trigger write) ----
                    ws = []
                    for c in range(NUM_CHUNKS):
                        t = xw.tile([P, Q * COUT], dt.bfloat16, tag=f"w{c}",
                                    name=f"w{c}")
                        engs[c % 2].dma_start(t[:], Wq[c * P:(c + 1) * P, :])
                        ws.append(t)

                    # ---- bias prep (PE transposes + bfin broadcast) ----
                    bT = route.tile([P, DT_ * NUM_CHUNKS], dt.float32)
                    for d in range(DT_):
                        ps_t = ps_pre.tile([P, NUM_CHUNKS], dt.float32,
                                           tag="pst")
                        nc.tensor.transpose(
                            ps_t[:], b_sb[:, d * P:(d + 1) * P],
                            ident[0:NUM_CHUNKS, 0:NUM_CHUNKS])
                        nc.scalar.copy(
                            bT[:, d * NUM_CHUNKS:(d + 1) * NUM_CHUNKS],
                            ps_t[:])
                    bfin_h = bfinp.tile([1, OUT_F], dt.bfloat16)
                    nc.vector.tensor_copy(bfin_h[:], bfin[:])
                    bfin_bc = bfinp.tile([P, OUT_F], dt.float32)
                    for o in range(OT):
                        sl = slice(o * 512, (o + 1) * 512)
                        ps_b = ps_pre.tile([P, 512], dt.float32, tag="psb")
                        nc.tensor.matmul(ps_b[:], ones_k1h[:], bfin_h[:, sl],
                                         start=True, stop=True)
                        nc.vector.tensor_copy(bfin_bc[:, sl], ps_b[:])

                    # ---- L1 for ALL 8 chunks (runs inside the AR window);
                    #      each finished chunk streams to DRAM over SWDGE --
                    with tc.tile_pool(name="ps_h", bufs=4,
                                      space="PSUM") as ps_h:
                        for c in range(NUM_CHUNKS):
                            for d in range(DT_):
                                ph = ps_h.tile([P, BS], dt.float32, tag="ph",
                                               name=f"ph{c}_{d}")
                                for j in range(Q):
                                    nc.tensor.matmul(
                                        ph[:],
                                        ws[c][:, j * COUT + d * P:
                                              j * COUT + (d + 1) * P],
                                        xs[c][:, j * BS:(j + 1) * BS],
                                        start=(j == 0), stop=(j == Q - 1))
                                nc.scalar.activation(
                                    hT[c][:, d * BS:(d + 1) * BS], ph[:],
                                    mybir.ActivationFunctionType.Identity,
                                    bias=bT[:, d * NUM_CHUNKS + c:
                                            d * NUM_CHUNKS + c + 1])
                            nc.gpsimd.dma_start(
                                hT_d[c * P:(c + 1) * P, :], hT[c][:])

            # ---------------- routing (post-AR) ----------------
            with tc.spectator_scope("route"):
                with tc.tile_pool(name="ps_rt", bufs=1,
                                  space="PSUM") as ps_rt:
                    act_g16 = route.tile([1, 2 * NUM_CHUNKS], dt.float32)
                    nc.gpsimd.dma_start(act_g16[:], cc_out[:])
                    act_g = route.tile([1, NUM_CHUNKS], dt.float32)
                    nc.vector.tensor_tensor(
                        out=act_g[:], in0=act_g16[0:1, 0:NUM_CHUNKS],
                        in1=act_g16[0:1, NUM_CHUNKS:2 * NUM_CHUNKS],
                        op=mybir.AluOpType.add)

                    maxv = route.tile([1, NUM_CHUNKS], dt.float32)
                    maxi = route.tile([1, NUM_CHUNKS], dt.uint32)
                    nc.vector.max(maxv[:], act_g[:])
                    nc.vector.max_index(maxi[:], maxv[:], act_g[:])
                    maxi_f = route.tile([1, NUM_CHUNKS], dt.float32)
                    nc.vector.tensor_copy(maxi_f[:], maxi[:])

                    # bcast[p, j] = idx[j] on every partition (K=1 matmul)
                    bc_ps = ps_rt.tile([P, NUM_CHUNKS], dt.float32, tag="psc")
                    nc.tensor.matmul(bc_ps[:], ones_k1[:], maxi_f[:],
                                     start=True, stop=True)

                    # gather offsets: off[p, s] = sel_s*128 + p (read the
                    # broadcast straight out of PSUM; fused mul-add)
                    off_f = route.tile([P, TOP_K], dt.float32)
                    for s in range(TOP_K):
                        nc.vector.scalar_tensor_tensor(
                            out=off_f[:, s:s + 1], in0=bc_ps[:, s:s + 1],
                            scalar=128.0, in1=C_Pf[:],
                            op0=mybir.AluOpType.mult,
                            op1=mybir.AluOpType.add)
                    off = route.tile([P, TOP_K], dt.int32)
                    nc.vector.tensor_copy(off[:], off_f[:])

                    # ---- PE warmup during route/gather (act_g-gated) ----
                    warm_rhs = route.tile([1, 256], dt.bfloat16)
                    nc.vector.tensor_scalar(
                        warm_rhs[:], bfin_h[0:1, 0:256],
                        act_g[0:1, 0:1], scalar2=None,
                        op0=mybir.AluOpType.add)
                    for wi in range(10):
                        ps_w = ps_rt.tile([P, 256], dt.float32, tag="psw")
                        nc.tensor.matmul(ps_w[:], ones_k1h[:], warm_rhs[:],
                                         start=True, stop=True)

            # ---------------- gather selected hT + L2 ----------------
            with tc.spectator_scope("l2"):
                with tc.tile_pool(name="wfs", bufs=1) as wfs, \
                     tc.tile_pool(name="hsel_p", bufs=1) as hsel_p, \
                     tc.tile_pool(name="outs", bufs=1) as outs, \
                     tc.tile_pool(name="ps_o", bufs=8, space="PSUM") as ps_o:
                    # hsel[s][p, d*512+b] = hT[sel_s][p, d*512+b]
                    hsel = [hsel_p.tile([P, DT_ * BS], dt.bfloat16,
                                        tag=f"hs{s}", name=f"hs{s}")
                            for s in range(TOP_K)]
                    # 4 indirect DMAs in flight (a single SWDGE gather op
                    # tops out around ~45GB/s)
                    gis = []
                    HW = DT_ * BS // 2
                    for s in range(TOP_K):
                        for hh in range(2):
                            gi = nc.gpsimd.indirect_dma_start(
                                out=hsel[s][:, hh * HW:(hh + 1) * HW],
                                out_offset=None,
                                in_=hT_d[:],
                                in_offset=bass.IndirectOffsetOnAxis(
                                    ap=off[:, s:s + 1], axis=0),
                                element_offset=hh * HW)
                            gis.append(gi)

                    # FIFO blockers: keep the W_final slabs off the rings
                    # until the collective completes (explicit deps — the
                    # scheduler otherwise hoists some wfo loads into the AR
                    # window, inflating the collective).
                    blk0 = route.tile([1, 2 * NUM_CHUNKS], dt.float32)
                    blk1 = route.tile([1, 2 * NUM_CHUNKS], dt.float32)
                    bi0 = nc.sync.dma_start(blk0[:], cc_out[0:1, :])
                    bi1 = nc.scalar.dma_start(blk1[:], cc_out[0:1, :])
                    wfo = []
                    for o in range(OT):
                        t = wfs.tile([P, OUT_F], dt.bfloat16, tag=f"wf{o}",
                                     name=f"wf{o}")
                        di = engs[o % 2].dma_start(
                            t[:], Wfr[o * P:(o + 1) * P, :])
                        add_dep_helper(di.ins, (bi0 if o % 2 == 0 else bi1).ins,
                                       sync=False,
                                       reason="wfo loads after AR blocker")
                        if o >= 2:
                            # leave HBM bandwidth to the hT gathers; these
                            # tiles still arrive well ahead of L2's o-loop
                            add_dep_helper(di.ins, gis[-1].ins, sync=True,
                                           reason="late wfo after hT gathers")
                        wfo.append(t)

                    # ---- L2: o-outer so wfo streams; out in quarter-rows --
                    orow = [outs.tile([P, OUT_F], dt.bfloat16, tag=f"or{bt}",
                                      name=f"or{bt}") for bt in range(BT)]
                    for o in range(OT):
                        osl = slice(o * 512, (o + 1) * 512)
                        po = [ps_o.tile([P, 512], dt.float32, tag="po",
                                        name=f"po{o}_{bt}")
                              for bt in range(BT)]
                        for kf in range(KF):
                            s, d = divmod(kf, DT_)
                            for bt in range(BT):
                                nc.tensor.matmul(
                                    po[bt][:],
                                    hsel[s][:, d * BS + bt * P:
                                            d * BS + (bt + 1) * P],
                                    wfo[o][:, kf * 512:(kf + 1) * 512],
                                    start=(kf == 0), stop=(kf == KF - 1))
                        for bt in range(BT):
                            nc.vector.tensor_tensor(
                                out=orow[bt][:, osl], in0=po[bt][:],
                                in1=bfin_bc[:, osl], op=mybir.AluOpType.add)
                        if o % 2 == 1:
                            qsl = slice((o - 1) * 512, (o + 1) * 512)
                            for bt in range(BT):
                                engs[bt % 2].dma_start(
                                    out[bt * P:(bt + 1) * P, qsl],
                                    orow[bt][:, qsl])
    nc.compile()
    return nc


def kernel(x, W_chunks, b_chunks, W_final, b_final):
    bf16 = ml_dtypes.bfloat16
    x = np.asarray(x, dtype=np.float32).astype(bf16)
    W_chunks = np.asarray(W_chunks, dtype=np.float32).astype(bf16)
    W_final = np.asarray(W_final, dtype=np.float32).astype(bf16)
    b_chunks = np.ascontiguousarray(np.asarray(b_chunks, dtype=np.float32))
    b_final = np.ascontiguousarray(
        np.asarray(b_final, dtype=np.float32).reshape(1, OUT_F))

    # o-major repack of W_final: Wfr[o*128+p, kf*512+n] = Wf[kf*128+p, o*512+n]
    Wfr = np.ascontiguousarray(
        W_final.reshape(KF, P, OT, 512).transpose(2, 1, 0, 3)
        .reshape(OT * P, KF * 512))

    if "nc" not in _cache:
        _cache["nc"] = _build()
    nc = _cache["nc"]

    in_maps = [{
        "xT_shard": np.ascontiguousarray(x[c * BS:(c + 1) * BS].T),
        "W_chunks": W_chunks,
        "b_chunks": b_chunks,
        "W_final_r": Wfr,
        "b_final": b_final,
    } for c in range(N_CORES)]

    res = run_bass_kernel_spmd(nc, in_maps, core_ids=list(range(N_CORES)))
    kernel.last_result = res
    return np.concatenate(
        [res.results[c]["out_shard"].astype(np.float32)
         for c in range(N_CORES)], axis=0)


kernel.last_result = None
